# revision 12
# baseline (speedup 1.0000x reference)
"""Trainium2 Bass kernel for nn_CNN2_P (dense CNN + MLP head).

Pure data-parallel over 8 NeuronCores: batch 2048 -> 256 per core, all
weights replicated. Host-side prep re-tiles weights into PE-friendly
layouts and casts to bf16; the device kernel runs conv1/2/3 as
accumulating matmuls (channels on partitions), keeps conv3 output
resident in SBUF (l-major, so fc1's moving operand is contiguous),
then streams fc1 weights from HBM (prefetched during the conv phase)
while accumulating fc1 in PSUM, and finishes with a flipped fc2
(weights stationary, fc1 activations moving) plus a tiny PE transpose
for the [B,16] store. PSUM drains rotate across ACT/DVE/GpSimd so no
single engine stalls the PE.
"""

import os

import numpy as np
import ml_dtypes

import concourse.mybir as mybir
import concourse.bacc as bacc
import concourse.tile as tile
from concourse.bass_utils import run_bass_kernel_spmd

# Problem constants (hardcoded per contract).
CL, IL = 128, 64          # context length, instruction length
CH = 256                  # channels in all three convs
L1, L2, L3 = 127, 125, 123
F1, OUT = 1024, 16
BATCH = 2048
NCORES = 8

BF16 = ml_dtypes.bfloat16

_CACHE = {}


def _build_program(B_pc, G):
    """Emit the per-core Bass program. B_pc = samples per core, G = chunk."""
    bf = mybir.dt.bfloat16
    f32 = mybir.dt.float32
    nchunks = B_pc // G
    ngrp = G // 4          # 4-sample matmul groups per chunk
    NT = F1 // 128         # 8 fc1 row tiles
    NWF = 14               # wf1 stream pool depth (prefetched in conv phase)
    NSB = B_pc // 128      # 128-sample blocks

    nc = bacc.Bacc("TRN2", target_bir_lowering=False, debug=False)

    xa_d = nc.dram_tensor("xa", [nchunks, 128, G * L1], bf, kind="ExternalInput")
    wa_d = nc.dram_tensor("wa", [128, CH], bf, kind="ExternalInput")
    w2_d = nc.dram_tensor("w2", [128, 12 * 128], bf, kind="ExternalInput")
    w3_d = nc.dram_tensor("w3", [128, 12 * 128], bf, kind="ExternalInput")
    wf1_d = nc.dram_tensor("wf1", [2 * L3, 128, F1], bf, kind="ExternalInput")
    wf2_d = nc.dram_tensor("wf2", [128, NT * OUT], bf, kind="ExternalInput")
    ident_d = nc.dram_tensor("ident", [16, 16], f32, kind="ExternalInput")
    bias_d = nc.dram_tensor("bias", [128, 6 + NT + 1], f32, kind="ExternalInput")
    out_d = nc.dram_tensor("out", [B_pc, OUT], f32, kind="ExternalOutput")

    relu = mybir.ActivationFunctionType.Relu
    ident_fn = mybir.ActivationFunctionType.Identity
    add_op = mybir.AluOpType.add
    max_op = mybir.AluOpType.max

    drain_ctr = [0]

    def drain(out_ap, in_ap, bias_ap, eng=None):
        """relu(in + bias) -> out on the given engine ('act'/'dve')."""
        if eng is None:
            eng = 'act' if drain_ctr[0] % 2 == 0 else 'dve'
            drain_ctr[0] += 1
        if eng == 'act':
            nc.scalar.activation(out_ap, in_ap, relu, bias=bias_ap)
        else:
            nc.vector.tensor_scalar(out_ap, in_ap, bias_ap, 0.0, add_op, max_op)

    with tile.TileContext(nc) as tc:
        with tc.tile_pool(name="persist", bufs=1) as pp:
            wa_t = pp.tile([128, CH], bf, name="wa_t", tag="wa")
            nc.scalar.dma_start(out=wa_t[:], in_=wa_d.ap())
            bias_t = pp.tile([128, 6 + NT + 1], f32, name="bias_t", tag="bias")
            nc.scalar.dma_start(out=bias_t[:], in_=bias_d.ap())
            w2_t = pp.tile([128, 12 * 128], bf, name="w2_t", tag="w2")
            nc.scalar.dma_start(out=w2_t[:], in_=w2_d.ap())
            w3_t = pp.tile([128, 12 * 128], bf, name="w3_t", tag="w3")
            nc.scalar.dma_start(out=w3_t[:], in_=w3_d.ap())
            wf2_t = pp.tile([128, NT * OUT], bf, name="wf2_t", tag="wf2")
            nc.scalar.dma_start(out=wf2_t[:], in_=wf2_d.ap())
            ident_t = pp.tile([16, 16], f32, name="ident_t", tag="ident")
            nc.scalar.dma_start(out=ident_t[:], in_=ident_d.ap())
            # conv3 output, resident, l-major: y3[ct][p, l*B_pc + s]
            y3_t = [pp.tile([128, L3 * B_pc], bf, name=f"y3_{i}", tag=f"y3_{i}") for i in range(2)]
            # fc1 output (post-relu), n_t-major columns
            out1_t = pp.tile([128, NT * B_pc], bf, name="out1_t", tag="out1")

            # wf1 stream pool spans conv + fc1 phases so the first NWF
            # tiles can prefetch while convs run (DMA queues are idle).
            wfp = tc.alloc_tile_pool(name="wf1", bufs=NWF)
            wf_tiles = []
            dmae = [nc.sync, nc.scalar, nc.gpsimd]

            # ---- conv phase ----
            with (
                tc.tile_pool(name="xa", bufs=4) as xap,
                tc.tile_pool(name="y1", bufs=2) as y1p,
                tc.tile_pool(name="y2", bufs=1) as y2p,
                tc.tile_pool(name="cpsum", bufs=8, space="PSUM") as cps,
            ):
                for c in range(nchunks):
                    xat = xap.tile([128, G * L1], bf, name="xa_t", tag="xa")
                    if c == 0:
                        q = G * L1 // 8
                        for sl in range(8):
                            nc.sync.dma_start(
                                out=xat[:, sl * q:(sl + 1) * q],
                                in_=xa_d.ap()[c][:, sl * q:(sl + 1) * q])
                    else:
                        nc.sync.dma_start(out=xat[:], in_=xa_d.ap()[c])
                    y1t = [y1p.tile([128, G * L1], bf, name=f"y1t_{i}", tag=f"y1_{i}") for i in range(2)]
                    y2t = [y2p.tile([128, G * L2], bf, name=f"y2t_{i}", tag=f"y2_{i}") for i in range(2)]
                    y1v = [y1t[i][:].rearrange("p (s l) -> p s l", l=L1)
                           for i in range(2)]
                    y2v = [y2t[i][:].rearrange("p (s l) -> p s l", l=L2)
                           for i in range(2)]

                    def emit_c1(g, ct):
                        # conv1: augmented K=128 matmul, N = 4*L1
                        ps = cps.tile([128, 4 * L1], f32, name="cps1", tag="cps")
                        nc.tensor.matmul(
                            ps[:],
                            wa_t[:, ct * 128:(ct + 1) * 128],
                            xat[:, g * 4 * L1:(g + 1) * 4 * L1],
                            start=True, stop=True,
                        )
                        # ACT takes one conv1 drain per chunk, DVE the rest
                        drain(y1t[ct][:, g * 4 * L1:(g + 1) * 4 * L1], ps[:],
                              bias_t[:, ct:ct + 1],
                              eng='act' if (g == 0 and ct == 0) else 'dve')

                    def emit_c2(g, ct):
                        # conv2: 3x2 accumulating matmuls per (group, co_t)
                        ps = cps.tile([128, 4 * L2], f32, name="cps2", tag="cps")
                        for k in range(3):
                            for ci in range(2):
                                j = k * 4 + ci * 2 + ct
                                nc.tensor.matmul(
                                    ps[:],
                                    w2_t[:, j * 128:(j + 1) * 128],
                                    y1v[ci][:, 4 * g:4 * g + 4, k:k + L2],
                                    start=(k == 0 and ci == 0),
                                    stop=(k == 2 and ci == 1),
                                )
                        drain(y2t[ct][:, g * 4 * L2:(g + 1) * 4 * L2], ps[:],
                              bias_t[:, 2 + ct:3 + ct],
                              eng='act' if (g == 0 and ct == 0) else 'dve')

                    def emit_c3(g, ct):
                        # conv3: writes l-major into resident y3; the strided
                        # drain is split in halves so each rotates engines
                        s0 = c * G + 4 * g
                        ps = cps.tile([128, 4 * L3], f32, name="cps3", tag="cps")
                        for k in range(3):
                            for ci in range(2):
                                j = k * 4 + ci * 2 + ct
                                nc.tensor.matmul(
                                    ps[:],
                                    w3_t[:, j * 128:(j + 1) * 128],
                                    y2v[ci][:, 4 * g:4 * g + 4, k:k + L3],
                                    start=(k == 0 and ci == 0),
                                    stop=(k == 2 and ci == 1),
                                )
                        y3v = y3_t[ct][:].rearrange("p (l s) -> p s l", s=B_pc)
                        psv = ps[:].rearrange("p (s m) -> p s m", m=L3)
                        # strided halves all on ACT (it handles strides well)
                        drain(y3v[:, s0:s0 + 2, :], psv[:, 0:2, :],
                              bias_t[:, 4 + ct:5 + ct], eng='act')
                        drain(y3v[:, s0 + 2:s0 + 4, :], psv[:, 2:4, :],
                              bias_t[:, 4 + ct:5 + ct], eng='act')

                    for g in range(ngrp):
                        for ct in range(2):
                            emit_c1(g, ct)
                    for g in range(ngrp):
                        for ct in range(2):
                            emit_c2(g, ct)
                    for g in range(ngrp):
                        for ct in range(2):
                            emit_c3(g, ct)
                    # stagger wf1 prefetch: one tile per chunk
                    if c < NWF:
                        wt = wfp.tile([128, F1], bf, name="wf1_t", tag="wf1")
                        nc.scalar.dma_start(out=wt[:], in_=wf1_d.ap()[c])
                        wf_tiles.append(wt)

            # ---- fc1: stream weights, accumulate all of PSUM ----
            with tc.tile_pool(name="fpsum", bufs=1, space="PSUM") as fps:
                psf = [fps.tile([128, B_pc], f32, name=f"psf_{i}", tag=f"psf_{i}") for i in range(NT)]
                for ct in range(2):
                    for l in range(L3):
                        r = ct * L3 + l
                        if r < NWF:
                            wt = wf_tiles[r]
                        else:
                            wt = wfp.tile([128, F1], bf, name="wf1_t", tag="wf1")
                            dmae[r % 3].dma_start(out=wt[:], in_=wf1_d.ap()[r])
                        rhs = y3_t[ct][:, l * B_pc:(l + 1) * B_pc]
                        for nt in range(NT):
                            nc.tensor.matmul(
                                psf[nt],
                                wt[:, nt * 128:(nt + 1) * 128],
                                rhs,
                                start=(ct == 0 and l == 0),
                                stop=(ct == 1 and l == L3 - 1),
                            )
                for nt in range(NT):
                    drain(out1_t[:, nt * B_pc:(nt + 1) * B_pc], psf[nt],
                          bias_t[:, 6 + nt:7 + nt])
            wfp.release()

            # ---- fc2 flipped + bias + PE transpose + store ----
            with (
                tc.tile_pool(name="opsum", bufs=2, space="PSUM") as ops,
                tc.tile_pool(name="osb", bufs=2) as osb,
            ):
                po = ops.tile([16, B_pc], f32, name="po", tag="po")
                for nt in range(NT):
                    nc.tensor.matmul(
                        po[:],
                        wf2_t[:, nt * OUT:(nt + 1) * OUT],
                        out1_t[:, nt * B_pc:(nt + 1) * B_pc],
                        start=(nt == 0), stop=(nt == NT - 1),
                    )
                po2 = osb.tile([16, B_pc], f32, name="po2", tag="po2")
                nc.scalar.activation(po2[:], po[:], ident_fn,
                                     bias=bias_t[0:16, 6 + NT:7 + NT])
                for sb in range(NSB):
                    pt = ops.tile([128, OUT], f32, name="pt", tag="pt")
                    nc.tensor.matmul(
                        pt[:], po2[:, sb * 128:(sb + 1) * 128], ident_t[:],
                        is_transpose=True, start=True, stop=True,
                    )
                    ot = osb.tile([128, OUT], f32, name="ot", tag="ot")
                    if sb % 2 == 0:
                        nc.vector.tensor_copy(ot[:], pt[:])
                    else:
                        nc.scalar.copy(ot[:], pt[:])
                    nc.sync.dma_start(
                        out=out_d.ap()[sb * 128:(sb + 1) * 128, :], in_=ot[:])

    nc.compile()
    return nc


def _host_prep(x, w1, b1, w2, b2, w3, b3, wfc1, bfc1, wfc2, bfc2, B_pc, G):
    """Build per-core input maps (shared weight arrays built once)."""
    NT = F1 // 128
    nchunks = B_pc // G

    # Augmented conv1 input: rows 0..63 = x0 broadcast, 64..127 = xr[:, :, 1:]
    B = x.shape[0]
    xr = np.ascontiguousarray(x.reshape(B, CL, IL).transpose(0, 2, 1))  # [B, IL, CL]
    xa = np.empty((B, 128, L1), dtype=np.float32)
    xa[:, :IL, :] = xr[:, :, 0:1]
    xa[:, IL:, :] = xr[:, :, 1:]
    xa = xa.astype(BF16)

    # conv1 weights: watilde[r, c] = w1[c, r, 0] (r<64) else w1[c, r-64, 1]
    wa = np.concatenate([w1[:, :, 0].T, w1[:, :, 1].T], axis=0).astype(BF16)
    wa = np.ascontiguousarray(wa)  # [128, 256]

    def conv_tiles(w):
        # w [co, ci, k] -> [ci(128), j*128+co], j = k*4 + ci_t*2 + co_t
        t = w.reshape(2, 128, 2, 128, 3)  # [co_t, co, ci_t, ci, k]
        t = t.transpose(4, 2, 0, 3, 1)    # [k, ci_t, co_t, ci, co]
        t = t.reshape(12, 128, 128).transpose(1, 0, 2).reshape(128, 12 * 128)
        return np.ascontiguousarray(t.astype(BF16))

    w2sb = conv_tiles(w2)
    w3sb = conv_tiles(w3)

    # fc1 weights: wf1[ct*123+l][co, nt*128+n] = wfc1[nt*128+n, (ct*128+co)*123+l]
    t = wfc1.reshape(F1, 2, 128, L3)      # [n, co_t, co, l]
    t = t.transpose(1, 3, 2, 0)           # [co_t, l, co, n]
    wf1 = np.ascontiguousarray(t.reshape(2 * L3, 128, F1).astype(BF16))

    # fc2: wf2[f, nt*16+o] = wfc2[o, nt*128+f]
    t = wfc2.T.reshape(NT, 128, OUT).transpose(1, 0, 2).reshape(128, NT * OUT)
    wf2 = np.ascontiguousarray(t.astype(BF16))

    ident = np.ascontiguousarray(np.eye(16, dtype=np.float32))

    bias = np.zeros((128, 6 + NT + 1), dtype=np.float32)
    bias[:, 0:2] = b1.reshape(2, 128).T
    bias[:, 2:4] = b2.reshape(2, 128).T
    bias[:, 4:6] = b3.reshape(2, 128).T
    bias[:, 6:6 + NT] = bfc1.reshape(NT, 128).T
    bias[0:OUT, 6 + NT] = bfc2

    in_maps = []
    ncores = B // B_pc
    for ci in range(ncores):
        shard = xa[ci * B_pc:(ci + 1) * B_pc]            # [B_pc, 128, L1]
        shard = shard.reshape(nchunks, G, 128, L1).transpose(0, 2, 1, 3)
        shard = np.ascontiguousarray(shard).reshape(nchunks, 128, G * L1)
        in_maps.append({
            "xa": shard, "wa": wa, "w2": w2sb, "w3": w3sb,
            "wf1": wf1, "wf2": wf2, "ident": ident, "bias": bias,
        })
    return in_maps


def kernel(x, w1, b1, w2, b2, w3, b3, wfc1, bfc1, wfc2, bfc2):
    B_pc = BATCH // NCORES
    G = 16
    key = ("prog", B_pc, G)
    if key not in _CACHE:
        _CACHE[key] = _build_program(B_pc, G)
    nc = _CACHE[key]
    in_maps = _host_prep(
        np.asarray(x, dtype=np.float32), np.asarray(w1, dtype=np.float32),
        np.asarray(b1, dtype=np.float32), np.asarray(w2, dtype=np.float32),
        np.asarray(b2, dtype=np.float32), np.asarray(w3, dtype=np.float32),
        np.asarray(b3, dtype=np.float32), np.asarray(wfc1, dtype=np.float32),
        np.asarray(bfc1, dtype=np.float32), np.asarray(wfc2, dtype=np.float32),
        np.asarray(bfc2, dtype=np.float32), B_pc, G,
    )
    trace = bool(os.environ.get("KERNEL_TRACE"))
    res = run_bass_kernel_spmd(nc, in_maps, core_ids=list(range(NCORES)),
                               trace=trace)
    _CACHE["last_results"] = res
    return np.concatenate([res.results[i]["out"] for i in range(NCORES)], axis=0)


# revision 13
# speedup vs baseline: 1.0844x; 1.0844x over previous
"""Trainium2 Bass kernel for nn_CNN2_P (dense CNN + MLP head).

Pure data-parallel over 8 NeuronCores: batch 2048 -> 256 per core, all
weights replicated. Host-side prep re-tiles weights into PE-friendly
layouts and casts to bf16; the device kernel runs conv1/2/3 as
accumulating matmuls (channels on partitions), keeps conv3 output
resident in SBUF, then streams fc1 weights from HBM while accumulating
fc1 in PSUM, and finishes with fc2.
"""

import os

import numpy as np
import ml_dtypes

import concourse.mybir as mybir
import concourse.bacc as bacc
import concourse.tile as tile
from concourse.bass_utils import run_bass_kernel_spmd

# Problem constants (hardcoded per contract).
CL, IL = 128, 64          # context length, instruction length
CH = 256                  # channels in all three convs
L1, L2, L3 = 127, 125, 123
F1, OUT = 1024, 16
BATCH = 2048
NCORES = 8

BF16 = ml_dtypes.bfloat16

_CACHE = {}


def _build_program(B_pc, G, conv_f32=False, fc2_f32=False):
    """Emit the per-core Bass program. B_pc = samples per core, G = chunk."""
    bf = mybir.dt.bfloat16
    f32 = mybir.dt.float32
    cdt = f32 if conv_f32 else bf      # conv activations/weights dtype
    odt = f32 if fc2_f32 else bf       # fc2 operand dtype
    nchunks = B_pc // G
    ngrp = G // 4          # 4-sample matmul groups per chunk
    NT = F1 // 128         # 8 fc1 row tiles

    nc = bacc.Bacc("TRN2", target_bir_lowering=False, debug=False)

    xa_d = nc.dram_tensor("xa", [nchunks, 128, G * L1], cdt, kind="ExternalInput")
    wa_d = nc.dram_tensor("wa", [128, CH], cdt, kind="ExternalInput")
    w2_d = nc.dram_tensor("w2", [128, 12 * 128], cdt, kind="ExternalInput")
    w3_d = nc.dram_tensor("w3", [128, 12 * 128], cdt, kind="ExternalInput")
    wf1_d = nc.dram_tensor("wf1", [2 * L3, 128, F1], bf, kind="ExternalInput")
    wf2_d = nc.dram_tensor("wf2", [128, NT * OUT], odt, kind="ExternalInput")
    bias_d = nc.dram_tensor("bias", [128, 6 + NT + OUT], f32, kind="ExternalInput")
    out_d = nc.dram_tensor("out", [B_pc, OUT], f32, kind="ExternalOutput")

    relu = mybir.ActivationFunctionType.Relu
    add_op = mybir.AluOpType.add
    max_op = mybir.AluOpType.max

    drain_ctr = [0]

    def drain(out_ap, in_ap, bias_ap):
        """relu(in + bias) -> out, alternating ACT / DVE."""
        if drain_ctr[0] % 2 == 0:
            nc.scalar.activation(out_ap, in_ap, relu, bias=bias_ap)
        else:
            nc.vector.tensor_scalar(out_ap, in_ap, bias_ap, 0.0, add_op, max_op)
        drain_ctr[0] += 1

    with tile.TileContext(nc) as tc:
        with tc.tile_pool(name="persist", bufs=1) as pp:
            wa_t = pp.tile([128, CH], cdt, name="wa_t", tag="wa")
            nc.scalar.dma_start(out=wa_t[:], in_=wa_d.ap())
            bias_t = pp.tile([128, 6 + NT + OUT], f32, name="bias_t", tag="bias")
            nc.scalar.dma_start(out=bias_t[:], in_=bias_d.ap())
            w2_t = pp.tile([128, 12 * 128], cdt, name="w2_t", tag="w2")
            nc.scalar.dma_start(out=w2_t[:], in_=w2_d.ap())
            w3_t = pp.tile([128, 12 * 128], cdt, name="w3_t", tag="w3")
            nc.scalar.dma_start(out=w3_t[:], in_=w3_d.ap())
            wf2_t = pp.tile([128, NT * OUT], odt, name="wf2_t", tag="wf2")
            nc.scalar.dma_start(out=wf2_t[:], in_=wf2_d.ap())
            # conv3 output, resident, l-major: y3[ct][p, l*B_pc + s]
            y3_t = [pp.tile([128, L3 * B_pc], bf, name=f"y3_{i}", tag=f"y3_{i}") for i in range(2)]
            # fc1 output (post-relu), n_t-major columns
            out1_t = pp.tile([128, NT * B_pc], odt, name="out1_t", tag="out1")

            # ---- conv phase ----
            with (
                tc.tile_pool(name="xa", bufs=4) as xap,
                tc.tile_pool(name="y1", bufs=2) as y1p,
                tc.tile_pool(name="y2", bufs=1) as y2p,
                tc.tile_pool(name="cpsum", bufs=8, space="PSUM") as cps,
            ):
                for c in range(nchunks):
                    xat = xap.tile([128, G * L1], cdt, name="xa_t", tag="xa")
                    if c == 0:
                        q = G * L1 // 8
                        for sl in range(8):
                            nc.sync.dma_start(
                                out=xat[:, sl * q:(sl + 1) * q],
                                in_=xa_d.ap()[c][:, sl * q:(sl + 1) * q])
                    else:
                        nc.sync.dma_start(out=xat[:], in_=xa_d.ap()[c])
                    y1t = [y1p.tile([128, G * L1], cdt, name=f"y1t_{i}", tag=f"y1_{i}") for i in range(2)]
                    y2t = [y2p.tile([128, G * L2], cdt, name=f"y2t_{i}", tag=f"y2_{i}") for i in range(2)]
                    y1v = [y1t[i][:].rearrange("p (s l) -> p s l", l=L1)
                           for i in range(2)]
                    y2v = [y2t[i][:].rearrange("p (s l) -> p s l", l=L2)
                           for i in range(2)]

                    def emit_c1(g, ct):
                        # conv1: augmented K=128 matmul, N = 4*L1
                        ps = cps.tile([128, 4 * L1], f32, name="cps1", tag="cps")
                        nc.tensor.matmul(
                            ps[:],
                            wa_t[:, ct * 128:(ct + 1) * 128],
                            xat[:, g * 4 * L1:(g + 1) * 4 * L1],
                            start=True, stop=True,
                        )
                        drain(y1t[ct][:, g * 4 * L1:(g + 1) * 4 * L1], ps[:],
                              bias_t[:, ct:ct + 1])

                    def emit_c2(g, ct):
                        # conv2: 3x2 accumulating matmuls per (group, co_t)
                        ps = cps.tile([128, 4 * L2], f32, name="cps2", tag="cps")
                        for k in range(3):
                            for ci in range(2):
                                j = k * 4 + ci * 2 + ct
                                nc.tensor.matmul(
                                    ps[:],
                                    w2_t[:, j * 128:(j + 1) * 128],
                                    y1v[ci][:, 4 * g:4 * g + 4, k:k + L2],
                                    start=(k == 0 and ci == 0),
                                    stop=(k == 2 and ci == 1),
                                )
                        drain(y2t[ct][:, g * 4 * L2:(g + 1) * 4 * L2], ps[:],
                              bias_t[:, 2 + ct:3 + ct])

                    def emit_c3(g, ct):
                        # conv3: writes l-major into resident y3
                        s0 = c * G + 4 * g
                        ps = cps.tile([128, 4 * L3], f32, name="cps3", tag="cps")
                        for k in range(3):
                            for ci in range(2):
                                j = k * 4 + ci * 2 + ct
                                nc.tensor.matmul(
                                    ps[:],
                                    w3_t[:, j * 128:(j + 1) * 128],
                                    y2v[ci][:, 4 * g:4 * g + 4, k:k + L3],
                                    start=(k == 0 and ci == 0),
                                    stop=(k == 2 and ci == 1),
                                )
                        y3v = y3_t[ct][:].rearrange("p (l s) -> p s l", s=B_pc)
                        psv = ps[:].rearrange("p (s m) -> p s m", m=L3)
                        # strided y3 drains are slow (~2.7us); split each
                        # across both engines to halve per-tile latency so
                        # PSUM slots recycle faster
                        nc.scalar.activation(y3v[:, s0:s0 + 2, :],
                                             psv[:, 0:2, :], relu,
                                             bias=bias_t[:, 4 + ct:5 + ct])
                        nc.vector.tensor_scalar(y3v[:, s0 + 2:s0 + 4, :],
                                                psv[:, 2:4, :],
                                                bias_t[:, 4 + ct:5 + ct],
                                                0.0, add_op, max_op)

                    for g in range(ngrp):
                        for ct in range(2):
                            emit_c1(g, ct)
                    for g in range(ngrp):
                        for ct in range(2):
                            emit_c2(g, ct)
                    for g in range(ngrp):
                        for ct in range(2):
                            emit_c3(g, ct)

            # ---- fc1: stream weights, accumulate all of PSUM ----
            with (
                tc.tile_pool(name="wf1", bufs=12) as wfp,
                tc.tile_pool(name="fpsum", bufs=1, space="PSUM") as fps,
            ):
                psf = [fps.tile([128, B_pc], f32, name=f"psf_{i}", tag=f"psf_{i}") for i in range(NT)]
                for ct in range(2):
                    for l in range(L3):
                        wt = wfp.tile([128, F1], bf, name="wf1_t", tag="wf1")
                        eng = (nc.sync, nc.scalar, nc.gpsimd)[(ct * L3 + l) % 3]
                        eng.dma_start(out=wt[:], in_=wf1_d.ap()[ct * L3 + l])
                        rhs = y3_t[ct][:, l * B_pc:(l + 1) * B_pc]
                        for nt in range(NT):
                            nc.tensor.matmul(
                                psf[nt],
                                wt[:, nt * 128:(nt + 1) * 128],
                                rhs,
                                start=(ct == 0 and l == 0),
                                stop=(ct == 1 and l == L3 - 1),
                            )
                for nt in range(NT):
                    drain(out1_t[:, nt * B_pc:(nt + 1) * B_pc], psf[nt],
                          bias_t[:, 6 + nt:7 + nt])

            # ---- fc2 + bias + store ----
            with (
                tc.tile_pool(name="opsum", bufs=2, space="PSUM") as ops,
                tc.tile_pool(name="osb", bufs=2) as osb,
            ):
                for bh in range((B_pc + 127) // 128):
                    bw = min(128, B_pc - bh * 128)
                    ps = ops.tile([128, OUT], f32, name="ops_t", tag="ops")
                    for nt in range(NT):
                        nc.tensor.matmul(
                            ps[:bw, :],
                            out1_t[:, nt * B_pc + bh * 128: nt * B_pc + bh * 128 + bw],
                            wf2_t[:, nt * OUT:(nt + 1) * OUT],
                            start=(nt == 0),
                            stop=(nt == NT - 1),
                        )
                    ot = osb.tile([128, OUT], f32, name="osb_t", tag="osb")
                    nc.vector.tensor_tensor(
                        out=ot[:bw, :], in0=ps[:bw, :],
                        in1=bias_t[:bw, 6 + NT:6 + NT + OUT],
                        op=mybir.AluOpType.add,
                    )
                    nc.sync.dma_start(out=out_d.ap()[bh * 128:bh * 128 + bw, :],
                                      in_=ot[:bw, :])

    nc.compile()
    return nc


def _host_prep(x, w1, b1, w2, b2, w3, b3, wfc1, bfc1, wfc2, bfc2, B_pc, G,
               conv_f32=False, fc2_f32=False):
    CDT = np.float32 if conv_f32 else BF16
    ODT = np.float32 if fc2_f32 else BF16
    """Build per-core input maps (shared weight arrays built once)."""
    NT = F1 // 128
    nchunks = B_pc // G

    # Augmented conv1 input: rows 0..63 = x0 broadcast, 64..127 = xr[:, :, 1:]
    B = x.shape[0]
    xr = np.ascontiguousarray(x.reshape(B, CL, IL).transpose(0, 2, 1))  # [B, IL, CL]
    xa = np.empty((B, 128, L1), dtype=np.float32)
    xa[:, :IL, :] = xr[:, :, 0:1]
    xa[:, IL:, :] = xr[:, :, 1:]
    xa = xa.astype(CDT)

    # conv1 weights: watilde[r, c] = w1[c, r, 0] (r<64) else w1[c, r-64, 1]
    wa = np.concatenate([w1[:, :, 0].T, w1[:, :, 1].T], axis=0).astype(CDT)
    wa = np.ascontiguousarray(wa)  # [128, 256]

    def conv_tiles(w):
        # w [co, ci, k] -> [ci(128), j*128+co], j = k*4 + ci_t*2 + co_t
        t = w.reshape(2, 128, 2, 128, 3)  # [co_t, co, ci_t, ci, k]
        t = t.transpose(4, 2, 0, 3, 1)    # [k, ci_t, co_t, ci, co]
        t = t.reshape(12, 128, 128).transpose(1, 0, 2).reshape(128, 12 * 128)
        return np.ascontiguousarray(t.astype(CDT))

    w2sb = conv_tiles(w2)
    w3sb = conv_tiles(w3)

    # fc1 weights: wf1[ct*123+l][co, nt*128+n] = wfc1[nt*128+n, (ct*128+co)*123+l]
    t = wfc1.reshape(F1, 2, 128, L3)      # [n, co_t, co, l]
    t = t.transpose(1, 3, 2, 0)           # [co_t, l, co, n]
    wf1 = np.ascontiguousarray(t.reshape(2 * L3, 128, F1).astype(BF16))

    # fc2: wf2[n, nt*16+o] = wfc2[o, nt*128+n]
    t = wfc2.T.reshape(NT, 128, OUT).transpose(1, 0, 2).reshape(128, NT * OUT)
    wf2 = np.ascontiguousarray(t.astype(ODT))

    bias = np.zeros((128, 6 + NT + OUT), dtype=np.float32)
    bias[:, 0:2] = b1.reshape(2, 128).T
    bias[:, 2:4] = b2.reshape(2, 128).T
    bias[:, 4:6] = b3.reshape(2, 128).T
    bias[:, 6:6 + NT] = bfc1.reshape(NT, 128).T
    bias[:, 6 + NT:] = bfc2[None, :]

    in_maps = []
    ncores = B // B_pc
    for ci in range(ncores):
        shard = xa[ci * B_pc:(ci + 1) * B_pc]            # [B_pc, 128, L1]
        shard = shard.reshape(nchunks, G, 128, L1).transpose(0, 2, 1, 3)
        shard = np.ascontiguousarray(shard).reshape(nchunks, 128, G * L1)
        in_maps.append({
            "xa": shard, "wa": wa, "w2": w2sb, "w3": w3sb,
            "wf1": wf1, "wf2": wf2, "bias": bias,
        })
    return in_maps


CONV_F32 = os.environ.get("KERNEL_CONV_F32", "0") == "1"
FC2_F32 = os.environ.get("KERNEL_FC2_F32", "0") == "1"


def kernel(x, w1, b1, w2, b2, w3, b3, wfc1, bfc1, wfc2, bfc2):
    B_pc = BATCH // NCORES
    G = 8 if CONV_F32 else 16
    key = ("prog", B_pc, G, CONV_F32, FC2_F32)
    if key not in _CACHE:
        _CACHE[key] = _build_program(B_pc, G, CONV_F32, FC2_F32)
    nc = _CACHE[key]
    in_maps = _host_prep(
        np.asarray(x, dtype=np.float32), np.asarray(w1, dtype=np.float32),
        np.asarray(b1, dtype=np.float32), np.asarray(w2, dtype=np.float32),
        np.asarray(b2, dtype=np.float32), np.asarray(w3, dtype=np.float32),
        np.asarray(b3, dtype=np.float32), np.asarray(wfc1, dtype=np.float32),
        np.asarray(bfc1, dtype=np.float32), np.asarray(wfc2, dtype=np.float32),
        np.asarray(bfc2, dtype=np.float32), B_pc, G, CONV_F32, FC2_F32,
    )
    trace = bool(os.environ.get("KERNEL_TRACE"))
    res = run_bass_kernel_spmd(nc, in_maps, core_ids=list(range(NCORES)),
                               trace=trace)
    _CACHE["last_results"] = res
    return np.concatenate([res.results[i]["out"] for i in range(NCORES)], axis=0)



# revision 14
# speedup vs baseline: 1.2785x; 1.1790x over previous
"""Trainium2 Bass kernel for nn_CNN2_P (dense CNN + MLP head).

Pure data-parallel over 8 NeuronCores: batch 2048 -> 256 per core, all
weights replicated. Host-side prep re-tiles weights into PE-friendly
layouts and casts to bf16. Device kernel: conv1/2/3 as accumulating
matmuls (channels on partitions) interleaved per 4-sample group so
PSUM drains spread evenly across ACT/DVE; conv3 output stays resident
in SBUF l-major so fc1's moving operand is contiguous. fc1 streams
weight rows from HBM (first tiles prefetched through the sync-queue
FIFO behind the xa loads, pacing them at conv speed) while
accumulating all 8 PSUM banks. fc2 is flipped (weights stationary,
fc1 activations moving, N=256) finishing with a tiny f32 PE transpose
for the [B,16] store.
"""

import os

import numpy as np
import ml_dtypes

import concourse.mybir as mybir
import concourse.bacc as bacc
import concourse.tile as tile
from concourse.bass_utils import run_bass_kernel_spmd

# Problem constants (hardcoded per contract).
CL, IL = 128, 64          # context length, instruction length
CH = 256                  # channels in all three convs
L1, L2, L3 = 127, 125, 123
F1, OUT = 1024, 16
BATCH = 2048
NCORES = 8

BF16 = ml_dtypes.bfloat16

_CACHE = {}


def _build_program(B_pc, G):
    """Emit the per-core Bass program. B_pc = samples per core, G = chunk."""
    bf = mybir.dt.bfloat16
    f32 = mybir.dt.float32
    nchunks = B_pc // G
    ngrp = G // 4          # 4-sample matmul groups per chunk
    NT = F1 // 128         # 8 fc1 row tiles
    NWF = 14               # wf1 stream pool depth (prefetched in conv phase)
    NSB = B_pc // 128      # 128-sample blocks

    nc = bacc.Bacc("TRN2", target_bir_lowering=False, debug=False)

    xa_d = nc.dram_tensor("xa", [nchunks, 128, G * L1], bf, kind="ExternalInput")
    wa_d = nc.dram_tensor("wa", [128, CH], bf, kind="ExternalInput")
    w2_d = nc.dram_tensor("w2", [128, 12 * 128], bf, kind="ExternalInput")
    w3_d = nc.dram_tensor("w3", [128, 12 * 128], bf, kind="ExternalInput")
    wf1_d = nc.dram_tensor("wf1", [2 * L3, 128, F1], bf, kind="ExternalInput")
    wf2_d = nc.dram_tensor("wf2", [128, NT * OUT], bf, kind="ExternalInput")
    ident_d = nc.dram_tensor("ident", [16, 16], f32, kind="ExternalInput")
    bias_d = nc.dram_tensor("bias", [128, 6 + NT + 1], f32, kind="ExternalInput")
    out_d = nc.dram_tensor("out", [B_pc, OUT], f32, kind="ExternalOutput")

    relu = mybir.ActivationFunctionType.Relu
    ident_fn = mybir.ActivationFunctionType.Identity
    add_op = mybir.AluOpType.add
    max_op = mybir.AluOpType.max

    drain_ctr = [0]

    def drain(out_ap, in_ap, bias_ap, eng=None):
        """relu(in + bias) -> out on the given engine ('act'/'dve')."""
        if eng is None:
            eng = 'act' if drain_ctr[0] % 2 == 0 else 'dve'
            drain_ctr[0] += 1
        if eng == 'act':
            nc.scalar.activation(out_ap, in_ap, relu, bias=bias_ap)
        else:
            nc.vector.tensor_scalar(out_ap, in_ap, bias_ap, 0.0, add_op, max_op)

    with tile.TileContext(nc) as tc:
        with tc.tile_pool(name="persist", bufs=1) as pp:
            wa_t = pp.tile([128, CH], bf, name="wa_t", tag="wa")
            nc.scalar.dma_start(out=wa_t[:], in_=wa_d.ap())
            bias_t = pp.tile([128, 6 + NT + 1], f32, name="bias_t", tag="bias")
            nc.scalar.dma_start(out=bias_t[:], in_=bias_d.ap())
            w2_t = pp.tile([128, 12 * 128], bf, name="w2_t", tag="w2")
            w3_t = pp.tile([128, 12 * 128], bf, name="w3_t", tag="w3")
            wf2_t = pp.tile([128, NT * OUT], bf, name="wf2_t", tag="wf2")
            ident_t = pp.tile([16, 16], f32, name="ident_t", tag="ident")
            # conv3 output, resident, l-major: y3[ct][p, l*B_pc + s]
            y3_t = [pp.tile([128, L3 * B_pc], bf, name=f"y3_{i}", tag=f"y3_{i}") for i in range(2)]
            # fc1 output (post-relu), n_t-major columns
            out1_t = pp.tile([128, NT * B_pc], bf, name="out1_t", tag="out1")

            # wf1 stream pool spans conv + fc1 phases so the first NWF
            # tiles can prefetch while convs run (DMA queues are idle).
            wfp = tc.alloc_tile_pool(name="wf1", bufs=NWF)
            wf_tiles = []
            dmae = [nc.sync, nc.scalar, nc.gpsimd]

            # ---- conv phase ----
            with (
                tc.tile_pool(name="xa", bufs=4) as xap,
                tc.tile_pool(name="y1", bufs=2) as y1p,
                tc.tile_pool(name="y2", bufs=1) as y2p,
                tc.tile_pool(name="cpsum", bufs=8, space="PSUM") as cps,
            ):
                for c in range(nchunks):
                    xat = xap.tile([128, G * L1], bf, name="xa_t", tag="xa")
                    if c == 0:
                        # slices 0-3 on sync, 4-7 on scalar (after wa+bias);
                        # the remaining persistent weights follow on scalar
                        q = G * L1 // 8
                        for sl in range(8):
                            eng = nc.sync if sl < 4 else nc.scalar
                            eng.dma_start(
                                out=xat[:, sl * q:(sl + 1) * q],
                                in_=xa_d.ap()[c][:, sl * q:(sl + 1) * q])
                        nc.scalar.dma_start(out=w2_t[:], in_=w2_d.ap())
                        nc.scalar.dma_start(out=w3_t[:], in_=w3_d.ap())
                        nc.scalar.dma_start(out=wf2_t[:], in_=wf2_d.ap())
                        nc.scalar.dma_start(out=ident_t[:], in_=ident_d.ap())
                    else:
                        nc.sync.dma_start(out=xat[:], in_=xa_d.ap()[c])
                    y1t = [y1p.tile([128, G * L1], bf, name=f"y1t_{i}", tag=f"y1_{i}") for i in range(2)]
                    y2t = [y2p.tile([128, G * L2], bf, name=f"y2t_{i}", tag=f"y2_{i}") for i in range(2)]
                    y1v = [y1t[i][:].rearrange("p (s l) -> p s l", l=L1)
                           for i in range(2)]
                    y2v = [y2t[i][:].rearrange("p (s l) -> p s l", l=L2)
                           for i in range(2)]

                    def emit_c1(g, ct):
                        # conv1: augmented K=128 matmul, N = 4*L1
                        ps = cps.tile([128, 4 * L1], f32, name="cps1", tag="cps")
                        nc.tensor.matmul(
                            ps[:],
                            wa_t[:, ct * 128:(ct + 1) * 128],
                            xat[:, g * 4 * L1:(g + 1) * 4 * L1],
                            start=True, stop=True,
                        )
                        drain(y1t[ct][:, g * 4 * L1:(g + 1) * 4 * L1], ps[:],
                              bias_t[:, ct:ct + 1],
                              eng='act' if ct == 0 else 'dve')

                    def emit_c2(g, ct):
                        # conv2: 3x2 accumulating matmuls per (group, co_t)
                        ps = cps.tile([128, 4 * L2], f32, name="cps2", tag="cps")
                        for k in range(3):
                            for ci in range(2):
                                j = k * 4 + ci * 2 + ct
                                nc.tensor.matmul(
                                    ps[:],
                                    w2_t[:, j * 128:(j + 1) * 128],
                                    y1v[ci][:, 4 * g:4 * g + 4, k:k + L2],
                                    start=(k == 0 and ci == 0),
                                    stop=(k == 2 and ci == 1),
                                )
                        drain(y2t[ct][:, g * 4 * L2:(g + 1) * 4 * L2], ps[:],
                              bias_t[:, 2 + ct:3 + ct], eng='act')

                    def emit_c3(g, ct):
                        # conv3: writes l-major into resident y3; the strided
                        # drain is split in halves so each rotates engines
                        s0 = c * G + 4 * g
                        ps = cps.tile([128, 4 * L3], f32, name="cps3", tag="cps")
                        for k in range(3):
                            for ci in range(2):
                                j = k * 4 + ci * 2 + ct
                                nc.tensor.matmul(
                                    ps[:],
                                    w3_t[:, j * 128:(j + 1) * 128],
                                    y2v[ci][:, 4 * g:4 * g + 4, k:k + L3],
                                    start=(k == 0 and ci == 0),
                                    stop=(k == 2 and ci == 1),
                                )
                        y3v = y3_t[ct][:].rearrange("p (l s) -> p s l", s=B_pc)
                        psv = ps[:].rearrange("p (s m) -> p s m", m=L3)
                        # strided drain halves in parallel on ACT + DVE
                        drain(y3v[:, s0:s0 + 2, :], psv[:, 0:2, :],
                              bias_t[:, 4 + ct:5 + ct], eng='act')
                        drain(y3v[:, s0 + 2:s0 + 4, :], psv[:, 2:4, :],
                              bias_t[:, 4 + ct:5 + ct], eng='dve')

                    for g in range(ngrp):
                        for ct in range(2):
                            emit_c1(g, ct)
                        for ct in range(2):
                            emit_c2(g, ct)
                        for ct in range(2):
                            emit_c3(g, ct)
                    # stagger wf1 prefetch: one tile per chunk
                    if c < NWF:
                        wt = wfp.tile([128, F1], bf, name="wf1_t", tag="wf1")
                        nc.sync.dma_start(out=wt[:], in_=wf1_d.ap()[c])
                        wf_tiles.append(wt)

            # ---- fc1: stream weights, accumulate all of PSUM ----
            with tc.tile_pool(name="fpsum", bufs=1, space="PSUM") as fps:
                psf = [fps.tile([128, B_pc], f32, name=f"psf_{i}", tag=f"psf_{i}") for i in range(NT)]
                for ct in range(2):
                    for l in range(L3):
                        r = ct * L3 + l
                        if r < NWF:
                            wt = wf_tiles[r]
                        else:
                            wt = wfp.tile([128, F1], bf, name="wf1_t", tag="wf1")
                            dmae[r % 3].dma_start(out=wt[:], in_=wf1_d.ap()[r])
                        rhs = y3_t[ct][:, l * B_pc:(l + 1) * B_pc]
                        for nt in range(NT):
                            nc.tensor.matmul(
                                psf[nt],
                                wt[:, nt * 128:(nt + 1) * 128],
                                rhs,
                                start=(ct == 0 and l == 0),
                                stop=(ct == 1 and l == L3 - 1),
                            )
                for nt in range(NT):
                    drain(out1_t[:, nt * B_pc:(nt + 1) * B_pc], psf[nt],
                          bias_t[:, 6 + nt:7 + nt])
            wfp.release()

            # ---- fc2 flipped + bias + PE transpose + store ----
            with (
                tc.tile_pool(name="opsum", bufs=2, space="PSUM") as ops,
                tc.tile_pool(name="osb", bufs=2) as osb,
            ):
                po = ops.tile([16, B_pc], f32, name="po", tag="po")
                for nt in range(NT):
                    nc.tensor.matmul(
                        po[:],
                        wf2_t[:, nt * OUT:(nt + 1) * OUT],
                        out1_t[:, nt * B_pc:(nt + 1) * B_pc],
                        start=(nt == 0), stop=(nt == NT - 1),
                    )
                po2 = osb.tile([16, B_pc], f32, name="po2", tag="po2")
                nc.scalar.activation(po2[:], po[:], ident_fn,
                                     bias=bias_t[0:16, 6 + NT:7 + NT])
                for sb in range(NSB):
                    pt = ops.tile([128, OUT], f32, name="pt", tag="pt")
                    nc.tensor.matmul(
                        pt[:], po2[:, sb * 128:(sb + 1) * 128], ident_t[:],
                        is_transpose=True, start=True, stop=True,
                    )
                    ot = osb.tile([128, OUT], f32, name="ot", tag="ot")
                    if sb % 2 == 0:
                        nc.vector.tensor_copy(ot[:], pt[:])
                    else:
                        nc.scalar.copy(ot[:], pt[:])
                    nc.sync.dma_start(
                        out=out_d.ap()[sb * 128:(sb + 1) * 128, :], in_=ot[:])

    nc.compile()
    return nc


def _host_prep(x, w1, b1, w2, b2, w3, b3, wfc1, bfc1, wfc2, bfc2, B_pc, G):
    """Build per-core input maps (shared weight arrays built once)."""
    NT = F1 // 128
    nchunks = B_pc // G

    # Augmented conv1 input: rows 0..63 = x0 broadcast, 64..127 = xr[:, :, 1:]
    B = x.shape[0]
    xr = np.ascontiguousarray(x.reshape(B, CL, IL).transpose(0, 2, 1))  # [B, IL, CL]
    xa = np.empty((B, 128, L1), dtype=np.float32)
    xa[:, :IL, :] = xr[:, :, 0:1]
    xa[:, IL:, :] = xr[:, :, 1:]
    xa = xa.astype(BF16)

    # conv1 weights: watilde[r, c] = w1[c, r, 0] (r<64) else w1[c, r-64, 1]
    wa = np.concatenate([w1[:, :, 0].T, w1[:, :, 1].T], axis=0).astype(BF16)
    wa = np.ascontiguousarray(wa)  # [128, 256]

    def conv_tiles(w):
        # w [co, ci, k] -> [ci(128), j*128+co], j = k*4 + ci_t*2 + co_t
        t = w.reshape(2, 128, 2, 128, 3)  # [co_t, co, ci_t, ci, k]
        t = t.transpose(4, 2, 0, 3, 1)    # [k, ci_t, co_t, ci, co]
        t = t.reshape(12, 128, 128).transpose(1, 0, 2).reshape(128, 12 * 128)
        return np.ascontiguousarray(t.astype(BF16))

    w2sb = conv_tiles(w2)
    w3sb = conv_tiles(w3)

    # fc1 weights: wf1[ct*123+l][co, nt*128+n] = wfc1[nt*128+n, (ct*128+co)*123+l]
    t = wfc1.reshape(F1, 2, 128, L3)      # [n, co_t, co, l]
    t = t.transpose(1, 3, 2, 0)           # [co_t, l, co, n]
    wf1 = np.ascontiguousarray(t.reshape(2 * L3, 128, F1).astype(BF16))

    # fc2: wf2[f, nt*16+o] = wfc2[o, nt*128+f]
    t = wfc2.T.reshape(NT, 128, OUT).transpose(1, 0, 2).reshape(128, NT * OUT)
    wf2 = np.ascontiguousarray(t.astype(BF16))

    ident = np.ascontiguousarray(np.eye(16, dtype=np.float32))

    bias = np.zeros((128, 6 + NT + 1), dtype=np.float32)
    bias[:, 0:2] = b1.reshape(2, 128).T
    bias[:, 2:4] = b2.reshape(2, 128).T
    bias[:, 4:6] = b3.reshape(2, 128).T
    bias[:, 6:6 + NT] = bfc1.reshape(NT, 128).T
    bias[0:OUT, 6 + NT] = bfc2

    in_maps = []
    ncores = B // B_pc
    for ci in range(ncores):
        shard = xa[ci * B_pc:(ci + 1) * B_pc]            # [B_pc, 128, L1]
        shard = shard.reshape(nchunks, G, 128, L1).transpose(0, 2, 1, 3)
        shard = np.ascontiguousarray(shard).reshape(nchunks, 128, G * L1)
        in_maps.append({
            "xa": shard, "wa": wa, "w2": w2sb, "w3": w3sb,
            "wf1": wf1, "wf2": wf2, "ident": ident, "bias": bias,
        })
    return in_maps


def kernel(x, w1, b1, w2, b2, w3, b3, wfc1, bfc1, wfc2, bfc2):
    B_pc = BATCH // NCORES
    G = 16
    key = ("prog", B_pc, G)
    if key not in _CACHE:
        _CACHE[key] = _build_program(B_pc, G)
    nc = _CACHE[key]
    in_maps = _host_prep(
        np.asarray(x, dtype=np.float32), np.asarray(w1, dtype=np.float32),
        np.asarray(b1, dtype=np.float32), np.asarray(w2, dtype=np.float32),
        np.asarray(b2, dtype=np.float32), np.asarray(w3, dtype=np.float32),
        np.asarray(b3, dtype=np.float32), np.asarray(wfc1, dtype=np.float32),
        np.asarray(bfc1, dtype=np.float32), np.asarray(wfc2, dtype=np.float32),
        np.asarray(bfc2, dtype=np.float32), B_pc, G,
    )
    trace = bool(os.environ.get("KERNEL_TRACE"))
    res = run_bass_kernel_spmd(nc, in_maps, core_ids=list(range(NCORES)),
                               trace=trace)
    _CACHE["last_results"] = res
    return np.concatenate([res.results[i]["out"] for i in range(NCORES)], axis=0)
